# revision 1
# baseline (speedup 1.0000x reference)
"""GCN layer (symmetric-normalized message passing + skip) on 8 Trainium2
NeuronCores via Bass/Tile.

    deg = bincount(src); dis = (deg>0) * rsqrt(max(deg,1))
    out = dis_dst * ( segsum_dst( dis_src * feats[src] ) @ Wm.T ) + bm
          + feats @ Ws.T + bs

Sharding: nodes split into 8 contiguous ranges of 12500 (dst owner); edges
partitioned by dst owner. Every core holds the full fp16 gather table.

v2 design (vs v1): fp16 message path with 256B gather rows; gathers spread
round-robin over 4 SWDGE queues (3.7x descriptor throughput vs one queue);
dst nodes in 512-wide superblocks so the one-hot scatter matmul accumulates
a [128,512] PSUM bank (4x fewer, wider DVE/PE ops), split into two
alternating PSUM accumulation chains merged at flush; both deg
normalizations fold into one per-edge weight w_e = dis[src_e]*dis[dst_e],
computed on device from host-supplied integer degree metadata and applied
per tile either by an ACT in-place scale of the gathered messages (3/4 of
tiles) or fused into the one-hot tensor_scalar (1/4), balancing the ACT and
DVE engines (~12us each per superblock), which pace the steady-state
pipeline. Gathers for superblock 0 issue immediately after the small gidx
load, ahead of the remaining input DMAs; each superblock's flush is
deferred one superblock so it queues behind the next tile stream.

Host prep supplies integer partitioning metadata only (sort orders, table
row permutation, per-edge degrees/slots); all float math runs on device.
"""

import numpy as np

P = 128
D = 128
NCORES = 8
N = 100000
E = 640000
NLOC = N // NCORES          # 12500
SBW = 512                   # superblock width (nodes)
NSB = 25                    # superblocks per core
NLOC_PAD = NSB * SBW        # 12800
NSUB = 4                    # gather sub-tables (int16 index limit)
SUB = 32704
TBQ = 7                     # tiles per (superblock, q) cell
CAP = TBQ * P               # 896 edge slots per cell = one gather sub-op
T_SB = NSUB * TBQ           # 28 tiles per superblock
T2 = NSB * T_SB             # 700 tiles per core
SOPW = CAP // 16            # 56 gidx cols per sub-op
PAD_SLOT = 9999.0


# ---------------------------------------------------------------- host prep

def _assign_q(src, dst, rng_tries=60):
    """Random q (sub-table) per node, balancing per-(core,superblock,q) edge
    counts under CAP. Returns q_of_node."""
    cell_base = (dst // NLOC) * NSB + (dst % NLOC) // SBW   # 0..199
    best = None
    for seed in range(rng_tries):
        rng = np.random.default_rng(seed)
        q = rng.integers(0, NSUB, N).astype(np.int32)
        cnt = np.bincount(cell_base * NSUB + q[src],
                          minlength=NCORES * NSB * NSUB)
        mx = cnt.max()
        if best is None or mx < best[1]:
            best = (q, mx)
        if mx <= CAP:
            break
    q, mx = best
    assert mx <= CAP, f"cell overflow {mx} > {CAP}; raise TBQ"
    return q


def _prep(feats, src, dst, wm, bm, ws, bs):
    n, d = feats.shape
    assert n == N and d == D
    src = np.asarray(src).astype(np.int64)
    dst = np.asarray(dst).astype(np.int64)
    feats = np.asarray(feats, dtype=np.float32)

    deg = np.bincount(src, minlength=n)
    q_of_node = _assign_q(src, dst)

    row_of_node = np.zeros(n, np.int64)
    for qq in range(NSUB):
        nodes = np.flatnonzero(q_of_node == qq)
        assert len(nodes) <= SUB, f"subtable {qq} overflow: {len(nodes)}"
        row_of_node[nodes] = qq * SUB + np.arange(len(nodes))
    feats_big = np.zeros((NSUB * SUB, D), np.float16)
    feats_big[row_of_node] = feats.astype(np.float16)

    wmT = np.ascontiguousarray(np.asarray(wm, np.float32).T).astype(np.float16)
    wsT = np.ascontiguousarray(np.asarray(ws, np.float32).T).astype(np.float16)
    bm = np.asarray(bm, np.float32).reshape(1, D)
    bs = np.asarray(bs, np.float32).reshape(1, D)
    iota16 = np.broadcast_to(np.arange(SBW, dtype=np.float16),
                             (P, SBW)).copy()

    in_maps = []
    for k in range(NCORES):
        m = (dst // NLOC) == k
        s, dg = src[m], dst[m]
        dl = dg - k * NLOC
        sb = dl // SBW
        slot = (dl % SBW).astype(np.float32)
        qe = q_of_node[s]
        lidx = (row_of_node[s] - qe * SUB).astype(np.int16)
        degS = deg[s].astype(np.int16)
        degD = deg[dg].astype(np.int16)

        cell = sb * NSUB + qe                       # 0..99
        order = np.argsort(cell, kind="stable")
        cell_s = cell[order]
        starts = np.searchsorted(cell_s, np.arange(NSB * NSUB + 1))
        counts = np.diff(starts)
        assert counts.max() <= CAP
        within = np.arange(len(cell_s)) - starts[cell_s]
        pos = cell_s * CAP + within

        nflat = NSB * NSUB * CAP
        flat_lidx = np.zeros(nflat, np.int16)
        flat_slot = np.full(nflat, PAD_SLOT, np.float32)
        flat_degS = np.zeros(nflat, np.int16)
        flat_degD = np.zeros(nflat, np.int16)
        flat_lidx[pos] = lidx[order]
        flat_slot[pos] = slot[order]
        flat_degS[pos] = degS[order]
        flat_degD[pos] = degD[order]

        # gidx: per cell (= sub-op) wrap 896 idx into [128, 56]
        a = flat_lidx.reshape(NSB * NSUB, SOPW, 16)      # [100, 56, 16]
        gidx = np.tile(a.transpose(0, 2, 1), (1, 8, 1))  # [100, 128, 56]
        gidx = np.ascontiguousarray(
            gidx.transpose(1, 0, 2).reshape(P, NSB * NSUB * SOPW))

        gslot = np.ascontiguousarray(flat_slot.reshape(T2, P).T)
        gdegS = np.ascontiguousarray(flat_degS.reshape(T2, P).T)
        gdegD = np.ascontiguousarray(flat_degD.reshape(T2, P).T)

        ft = np.zeros((P, NLOC_PAD), np.float16)
        ft[:, :NLOC] = feats[k * NLOC:(k + 1) * NLOC].T.astype(np.float16)

        in_maps.append({
            "gidx": gidx, "gslot": gslot, "gdegS": gdegS, "gdegD": gdegD,
            "featsT": ft, "feats_big": feats_big,
            "wmT": wmT, "wsT": wsT, "bm": bm, "bs": bs, "iota16": iota16,
        })
    return in_maps


# ------------------------------------------------------------- device kernel

def device_kernel(tc, outs, ins, cfg):
    import concourse.mybir as mybir

    nc = tc.nc
    f32 = mybir.dt.float32
    f16 = mybir.dt.float16
    i16 = mybir.dt.int16
    Op = mybir.AluOpType
    Act = mybir.ActivationFunctionType

    (out_d,) = outs
    (gidx_d, gslot_d, gdegS_d, gdegD_d, featsT_d, feats_big_d,
     wmT_d, wsT_d, bm_d, bs_d, iota_d) = ins

    abl = cfg.get("ABL", ())

    with (
        tc.tile_pool(name="sbuf", bufs=1) as sb,
        tc.tile_pool(name="smsg", bufs=2) as smsg,
        tc.tile_pool(name="soh", bufs=4) as soh,
        tc.tile_pool(name="srst", bufs=2) as srst,
        tc.tile_pool(name="sstg", bufs=4) as sstg,
        tc.tile_pool(name="psag", bufs=2, space="PSUM") as psag,
        tc.tile_pool(name="pslin", bufs=4, space="PSUM") as pslin,
    ):
        # ---------------- setup ----------------
        # gidx first: the sb0 gathers depend only on it (+ feats_big already
        # in HBM), so they issue under the remaining input loads + w compute.
        gidx = sb.tile([P, NSB * NSUB * SOPW], i16)
        nc.sync.dma_start(out=gidx[:], in_=gidx_d[:])

        qc_box = [0]
        msgs_ring = {}

        def issue_gathers(sbi):
            msgs = smsg.tile([P, T_SB * P], f16, tag="msgs")
            msgs_ring[sbi] = msgs
            if "gather" in abl:
                nc.vector.memset(msgs[:], 0.0)
                return
            for q in range(NSUB):
                cell = sbi * NSUB + q
                nc.gpsimd.dma_gather(
                    msgs[:, q * CAP:(q + 1) * CAP]
                    .rearrange("p (t e) -> p t e", e=D),
                    feats_big_d[q * SUB:(q + 1) * SUB, :],
                    gidx[:, cell * SOPW:(cell + 1) * SOPW],
                    CAP, CAP, D, queue_num=(q + sbi) % 4)
                qc_box[0] += 1

        issue_gathers(0)

        gslot = sb.tile([P, T2], f32)
        nc.sync.dma_start(out=gslot[:], in_=gslot_d[:])
        iota_t = sb.tile([P, SBW], f16)
        nc.sync.dma_start(out=iota_t[:], in_=iota_d[:])
        wmT = sb.tile([P, D], f16)
        nc.sync.dma_start(out=wmT[:], in_=wmT_d[:])
        wsT = sb.tile([P, D], f16)
        nc.sync.dma_start(out=wsT[:], in_=wsT_d[:])
        featsT = sb.tile([P, NLOC_PAD], f16)
        nc.sync.dma_start(out=featsT[:], in_=featsT_d[:])

        # bias16 = (bm + bs) as f16 row
        bmt = sb.tile([1, D], f32)
        nc.sync.dma_start(out=bmt[:], in_=bm_d[:])
        bst = sb.tile([1, D], f32)
        nc.sync.dma_start(out=bst[:], in_=bs_d[:])
        nc.vector.tensor_tensor(out=bmt[:], in0=bmt[:], in1=bst[:], op=Op.add)
        bias16 = sb.tile([1, D], f16)
        nc.vector.tensor_copy(out=bias16[:], in_=bmt[:])
        ones1 = sb.tile([1, P], f16)
        nc.vector.memset(ones1[:], 1.0)

        # wE[p, t] = dis(degS) * dis(degD) per edge, dis(x)=(x>0)*rsqrt(max(x,1))
        def dis_of(deg_d):
            di = sb.tile([P, T2], i16, tag="digi")
            nc.sync.dma_start(out=di[:], in_=deg_d[:])
            df = sb.tile([P, T2], f32, tag="digf")
            nc.vector.tensor_copy(out=df[:], in_=di[:])
            msk = sb.tile([P, T2], f32, tag="dmsk")
            nc.vector.tensor_scalar(out=msk[:], in0=df[:], scalar1=0.0,
                                    scalar2=None, op0=Op.is_gt)
            nc.vector.tensor_scalar(out=df[:], in0=df[:], scalar1=1.0,
                                    scalar2=None, op0=Op.max)
            rc = sb.tile([P, T2], f32, tag="drc")
            nc.vector.reciprocal(out=rc[:], in_=df[:])
            rt = sb.tile([P, T2], f32, tag="drt")
            nc.scalar.activation(out=rt[:], in_=rc[:], func=Act.Sqrt)
            w = sb.tile([P, T2], f32, tag=f"dw{id(deg_d)}")
            nc.vector.tensor_tensor(out=w[:], in0=rt[:], in1=msk[:],
                                    op=Op.mult)
            return w

        wS = dis_of(gdegS_d)
        wD = dis_of(gdegD_d)
        wE = sb.tile([P, T2], f32)
        nc.vector.tensor_tensor(out=wE[:], in0=wS[:], in1=wD[:], op=Op.mult)

        # ---------------- main loop ----------------
        def tiles_of(sbi):
            msgs = msgs_ring.pop(sbi)
            bankA = psag.tile([P, SBW], f32, tag="aggA", space="PSUM")
            bankB = psag.tile([P, SBW], f32, tag="aggB", space="PSUM")
            for t in range(T_SB):
                T0 = sbi * T_SB + t
                oh = soh.tile([P, SBW], f16, tag="oh")
                if "onehot" not in abl:
                    if t % 4 == 0:
                        nc.vector.tensor_scalar(
                            out=oh[:], in0=iota_t[:],
                            scalar1=gslot[:, T0:T0 + 1],
                            scalar2=wE[:, T0:T0 + 1],
                            op0=Op.is_equal, op1=Op.mult)
                    else:
                        if "scale" not in abl:
                            nc.scalar.activation(
                                out=msgs[:, t * P:(t + 1) * P],
                                in_=msgs[:, t * P:(t + 1) * P],
                                func=Act.Copy,
                                scale=wE[:, T0:T0 + 1])
                        nc.vector.tensor_scalar(
                            out=oh[:], in0=iota_t[:],
                            scalar1=gslot[:, T0:T0 + 1],
                            scalar2=None, op0=Op.is_equal)
                if "aggmm" in abl:
                    continue
                bank = bankA if t % 2 == 0 else bankB
                nc.tensor.matmul(
                    out=bank[:], lhsT=msgs[:, t * P:(t + 1) * P],
                    rhs=oh[:] if "onehot" not in abl else iota_t[:],
                    start=(t < 2), stop=(t >= T_SB - 2))
            return bankA, bankB

        def flush_of(sbi, bankA, bankB):
            if "aggmm" in abl or "flush" in abl:
                return
            rstT = srst.tile([P, SBW], f16, tag="rstT")
            nc.scalar.copy(out=rstT[:], in_=bankA[:])
            nc.vector.tensor_tensor(out=rstT[:], in0=bankB[:],
                                    in1=rstT[:], op=Op.add)
            for b in range(4):
                pmk = pslin.tile([P, D], f32, tag="pmk", space="PSUM")
                nc.tensor.matmul(out=pmk[:],
                                 lhsT=rstT[:, b * P:(b + 1) * P],
                                 rhs=wmT[:], start=True, stop=False)
                nc.tensor.matmul(out=pmk[:],
                                 lhsT=featsT[:, (sbi * 4 + b) * P:
                                             (sbi * 4 + b + 1) * P],
                                 rhs=wsT[:], start=False, stop=False)
                nc.tensor.matmul(out=pmk[:], lhsT=ones1[:], rhs=bias16[:],
                                 start=False, stop=True)
                stage = sstg.tile([P, D], f32, tag="stage")
                nc.scalar.copy(out=stage[:], in_=pmk[:])
                nc.sync.dma_start(
                    out=out_d[(sbi * 4 + b) * P:(sbi * 4 + b + 1) * P, :],
                    in_=stage[:])

        def body():
            if 0 not in msgs_ring:
                issue_gathers(0)
            # flush deferred one superblock so its ACT/DVE/PE ops queue
            # behind the next superblock's tile stream (no head-of-line
            # stalls at superblock boundaries).
            pending = None
            for sbi in range(NSB):
                if sbi + 1 < NSB:
                    issue_gathers(sbi + 1)
                banks = tiles_of(sbi)
                if pending is not None:
                    flush_of(*pending)
                pending = (sbi, *banks)
            flush_of(*pending)

        for _ in range(cfg.get("REPEAT", 1)):
            body()


# --------------------------------------------------------------- entry point

def _build_program(cfg):
    import concourse.bacc as bacc
    import concourse.mybir as mybir
    import concourse.tile as tile

    f32 = mybir.dt.float32
    f16 = mybir.dt.float16
    i16 = mybir.dt.int16

    nc = bacc.Bacc("TRN2", target_bir_lowering=False, debug=False,
                   enable_asserts=False, num_devices=NCORES,
                   num_swdge_queues=4)

    def inp(name, shape, dt):
        return nc.dram_tensor(name, shape, dt, kind="ExternalInput").ap()

    gidx = inp("gidx", [P, NSB * NSUB * SOPW], i16)
    gslot = inp("gslot", [P, T2], f32)
    gdegS = inp("gdegS", [P, T2], i16)
    gdegD = inp("gdegD", [P, T2], i16)
    featsT = inp("featsT", [P, NLOC_PAD], f16)
    feats_big = inp("feats_big", [NSUB * SUB, D], f16)
    wmT = inp("wmT", [P, D], f16)
    wsT = inp("wsT", [P, D], f16)
    bm = inp("bm", [1, D], f32)
    bs = inp("bs", [1, D], f32)
    iota16 = inp("iota16", [P, SBW], f16)
    out = nc.dram_tensor("out", [NLOC_PAD, D], f32, kind="ExternalOutput").ap()

    with tile.TileContext(nc) as tc:
        device_kernel(
            tc, [out],
            [gidx, gslot, gdegS, gdegD, featsT, feats_big, wmT, wsT,
             bm, bs, iota16],
            cfg)
    nc.compile()
    return nc


LAST_EXEC_NS = None


def kernel(feats, src, dst, linear_skip_weight, linear_skip_bias,
           linear_msg_weight, linear_msg_bias):
    global LAST_EXEC_NS
    import os

    from concourse.bass_utils import run_bass_kernel_spmd

    feats = np.asarray(feats)
    in_maps = _prep(feats, src, dst, linear_msg_weight, linear_msg_bias,
                    linear_skip_weight, linear_skip_bias)
    nc = _build_program({})
    trace = bool(int(os.environ.get("GCN_TRACE", "0")))
    res = run_bass_kernel_spmd(nc, in_maps, core_ids=list(range(NCORES)),
                               trace=trace)
    LAST_EXEC_NS = res.exec_time_ns
    if res.instructions_and_trace is not None:
        print("trace:", res.instructions_and_trace[1])
    out = np.empty((N, D), np.float32)
    for k in range(NCORES):
        out[k * NLOC:(k + 1) * NLOC] = res.results[k]["out"][:NLOC]
    return out

